# revision 2
# baseline (speedup 1.0000x reference)
"""
LutLinear (BCQ 3-bit, group=128) matvec kernel for 8 Trainium2 NeuronCores.

y = x @ W + bias,  W[k,n] = sum_b alpha[g(k),b,n]*B[k,b,n] + q_bias[g(k),n]
with B = 2*bit-1 from bit-packed binaryWeight [K//32, WBIT, N] (bit j of word
i <-> input index 32*i + j).

Strategy (tensor-parallel over N, 512 columns per core):
  y[n] = sum_{g,b} alpha[g,b,n] * t[g,b,n] + sum_g s_g * q_bias[g,n] + bias[n]
  t[g,b,n] = sum_{k in g} x_k * B[k,b,n];   s_g = sum_{k in g} x_k  (host)

On device the packed words (w = k//32 on the 128 SBUF partitions) are expanded
to fp8 "+-1" values with 16 tensor_scalar passes (DVE + GPSIMD split):
  dep_r = ((words << (7-r)) | 0x38383838) & 0xB8B8B8B8      r = 0..7
Each output byte lane L of pass r is the fp8 encoding of -B for input bit
j = r + 8L  (bit=1 -> 0xB8 = -1.0, bit=0 -> 0x38 = +1.0).  The PE contracts
over words with 32x accumulating matmuls (one per j) whose stationary operand
is a block-diagonal [128 x 32] fp16 matrix holding -x[32w+j] (negations
cancel), with byte-lane L mapped to PE column-group L (tile_position) so four
matmuls stream concurrently.  The alpha-weighted reduction over (g,b) and the
q_bias/bias terms are folded into a second accumulating matmul chain.
"""

import os
import sys

import numpy as np

sys.path.insert(0, "/opt/trn_rl_repo")

import concourse.bacc as bacc
import concourse.bass as bass
import concourse.mybir as mybir
import concourse.tile as tile
from concourse.bass_utils import run_bass_kernel_spmd

K, N, WBIT, GROUP = 4096, 4096, 3, 128
NCORES = 8
NS = N // NCORES          # 512 output columns per core
NG = K // GROUP           # 32 groups
W = K // 32               # 128 packed words along K
F = WBIT * NS             # 1536 free elements (b, n) per partition
FP8_OR = 0x38383838       # 1.0 in every fp8e4m3 byte lane
# single-instruction deposit: dep_r = (words & (0x01010101<<r)) [| 0x38383838]
# byte value pairs (bit=0 -> a_r, bit=1 -> a_r + d_r); d folded into x on
# host, a folded into a per-partition correction subtracted in the tail.
OR_RS = (0, 1, 2, 7)
D_R = {0: 0.125, 1: 0.25, 2: 0.5, 3: 2.0**-6, 4: 2.0**-5, 5: 2.0**-3,
       6: 2.0, 7: -2.0}
DMA_SPLIT = 4             # words DMA chunks (along free dim) for overlap

_CACHE = {}


def _declare_io(nc):
    f32 = mybir.dt.float32
    i32 = mybir.dt.int32
    fp16 = mybir.dt.float16
    d = {}
    d["words"] = nc.declare_dram_parameter("words", [W, F], i32, isOutput=False)
    d["xb"] = nc.declare_dram_parameter("xb", [W, 32 * 32], fp16, isOutput=False)
    d["alpha_rep"] = nc.declare_dram_parameter(
        "alpha_rep", [128, F], f32, isOutput=False)
    d["qbias"] = nc.declare_dram_parameter("qbias", [NG, NS], f32, isOutput=False)
    d["bias"] = nc.declare_dram_parameter("bias", [1, NS], f32, isOutput=False)
    d["s"] = nc.declare_dram_parameter("s", [NG, 1], f32, isOutput=False)
    d["corr"] = nc.declare_dram_parameter("corr", [128, 1], f32, isOutput=False)
    d["y"] = nc.declare_dram_parameter("y", [1, NS], f32, isOutput=True)
    return d


def _emit_body(nc, tiles, dram, rep):
    f8 = mybir.dt.float8e4
    words, xb, alpha, qb, bias_t, s_t, ones, dep, prod, y_sb, corr_t = tiles

    fchunk = F // DMA_SPLIT
    for c in range(DMA_SPLIT):
        sl = slice(c * fchunk, (c + 1) * fchunk)
        nc.sync.dma_start(words[:, sl], dram["words"][:, sl])
    nc.sync.dma_start(xb[:], dram["xb"][:])
    nc.sync.dma_start(alpha[:], dram["alpha_rep"][:])
    nc.sync.dma_start(qb[:], dram["qbias"][:])
    nc.sync.dma_start(bias_t[:], dram["bias"][:])
    nc.sync.dma_start(s_t[:], dram["s"][:])
    nc.sync.dma_start(corr_t[:], dram["corr"][:])

    # bit deposit: one DVE tensor_scalar per r
    for r in range(8):
        blk = dep[:, r * F:(r + 1) * F]
        mask = (0x01010101 << r) & 0xFFFFFFFF
        if mask >= 1 << 31:
            mask -= 1 << 32
        if r in OR_RS:
            nc.vector.tensor_scalar(
                blk, words[:], mask, FP8_OR,
                op0=mybir.AluOpType.bitwise_and,
                op1=mybir.AluOpType.bitwise_or,
            )
        else:
            nc.vector.tensor_scalar(
                blk, words[:], mask, None,
                op0=mybir.AluOpType.bitwise_and,
            )

    # fp8 view: free index = r*4F + idx*4 + L, idx = b*NS + n,
    # value = a_r + d_r * bit[32w + r + 8L, b, n]
    dep8 = dep[:].bitcast(f8).rearrange("w (r i l) -> w r i l", r=8, i=F, l=4)

    tb_all, yp = tiles_psum(rep)
    for r in range(8):
        for b in range(WBIT):
            for L in range(4):
                j = r + 8 * L
                lhsT = xb[:, j * 32:(j + 1) * 32]
                rhs = dep8[:, r, b * NS:(b + 1) * NS, L]
                nc.tensor.matmul(
                    tb_all[32 * L:32 * (L + 1), b * NS:(b + 1) * NS],
                    lhsT,
                    rhs,
                    start=(r == 0),
                    stop=(r == 7),
                    tile_position=(0, 32 * L),
                    skip_group_check=True,
                )

    nc.vector.scalar_tensor_tensor(
        prod[:], tb_all[:], corr_t[:], alpha[:],
        op0=mybir.AluOpType.subtract, op1=mybir.AluOpType.mult)
    for b in range(WBIT):
        nc.tensor.matmul(
            yp[:], ones[:], prod[:, b * NS:(b + 1) * NS],
            start=(b == 0), stop=False,
        )
    nc.tensor.matmul(yp[:], s_t[:], qb[:], start=False, stop=False)
    nc.tensor.matmul(yp[:], ones[:1, :], bias_t[:], start=False, stop=True)

    nc.scalar.copy(y_sb[:], yp[:])
    nc.sync.dma_start(dram["y"][:], y_sb[:])


def tiles_psum(rep):
    return _CACHE["psum_tiles"]


def _build_program(reps=1):
    nc = bacc.Bacc(None, target_bir_lowering=False, debug=False)
    f32 = mybir.dt.float32
    i32 = mybir.dt.int32
    fp16 = mybir.dt.float16
    dram = _declare_io(nc)

    with tile.TileContext(nc) as tc:
        with (
            tc.tile_pool(name="pool", bufs=1) as pool,
            tc.tile_pool(name="psum", bufs=1, space="PSUM") as psum,
        ):
            words = pool.tile([W, F], i32, name="words_sb")
            xb = pool.tile([W, 32 * 32], fp16, name="xb_sb")
            alpha = pool.tile([128, F], f32, name="alpha_sb")
            qb = pool.tile([NG, NS], f32, name="qb_sb")
            bias_t = pool.tile([1, NS], f32, name="bias_sb")
            s_t = pool.tile([NG, 1], f32, name="s_sb")
            ones = pool.tile([128, 1], f32, name="ones_sb")
            dep = pool.tile([W, 8 * F], i32, name="dep_sb")
            prod = pool.tile([128, F], f32, name="prod_sb")
            y_sb = pool.tile([1, NS], f32, name="y_out_sb")
            corr_t = pool.tile([128, 1], f32, name="corr_sb")
            _CACHE["psum_tiles"] = (
                psum.tile([128, F], f32, name="tb_all"),
                psum.tile([1, NS], f32, name="yp"),
            )
            nc.vector.memset(ones[:], 1.0)
            tiles = (words, xb, alpha, qb, bias_t, s_t, ones, dep, prod,
                     y_sb, corr_t)
            for rep in range(reps):
                _emit_body(nc, tiles, dram, rep)

    nc.compile()
    return nc


def _host_prep(x, binaryWeight, alpha, q_bias, bias):
    """Build the 8 per-core input maps (pure layout work + tiny x sums)."""
    x = np.asarray(x, np.float32).reshape(K)
    bw = np.asarray(binaryWeight)            # [W, WBIT, N] int32
    al = np.asarray(alpha, np.float32)       # [NG, WBIT, N]
    qb = np.asarray(q_bias, np.float32)      # [NG, N]
    bs = np.asarray(bias, np.float32)        # [N]

    xv = x.reshape(W, 32)                                # [w, j]
    dscale = np.array([D_R[j % 8] for j in range(32)], np.float32)
    blk = np.zeros((W, 32, 32), np.float16)              # [w, j, g]
    for g in range(32):
        blk[4 * g:4 * g + 4, :, g] = (
            xv[4 * g:4 * g + 4, :] / dscale[None, :]).astype(np.float16)
    corr = np.zeros((128, 1), np.float32)
    for L in range(4):
        for g in range(32):
            corr[32 * L + g, 0] = sum(
                blk[4 * g:4 * g + 4, r + 8 * L, g].astype(np.float32).sum()
                for r in OR_RS)
    s = x.reshape(NG, GROUP).sum(axis=1).astype(np.float32).reshape(NG, 1)
    asum = al.sum(axis=1)                                # [NG, N]

    in_maps = []
    for c in range(NCORES):
        n0 = c * NS
        words = np.ascontiguousarray(bw[:, :, n0:n0 + NS].reshape(W, F))
        alpha_s = 2.0 * al[:, :, n0:n0 + NS].reshape(NG, F)
        alpha_rep = np.ascontiguousarray(np.tile(alpha_s, (4, 1)))
        in_maps.append({
            "words": words.astype(np.int32),
            "xb": np.ascontiguousarray(blk.reshape(W, 32 * 32)),
            "alpha_rep": alpha_rep,
            "qbias": np.ascontiguousarray(
                qb[:, n0:n0 + NS] - asum[:, n0:n0 + NS]),
            "bias": np.ascontiguousarray(bs[n0:n0 + NS].reshape(1, NS)),
            "s": s,
            "corr": corr,
        })
    return in_maps


def kernel(x, binaryWeight, alpha, q_bias, bias, _trace=False):
    if "nc" not in _CACHE:
        _CACHE["nc"] = _build_program()
    nc = _CACHE["nc"]
    in_maps = _host_prep(x, binaryWeight, alpha, q_bias, bias)
    res = run_bass_kernel_spmd(nc, in_maps, list(range(NCORES)), trace=_trace)
    _CACHE["last_res"] = res
    _CACHE["last_exec_time_ns"] = res.exec_time_ns
    y = np.concatenate([res.results[c]["y"] for c in range(NCORES)], axis=1)
    return y.astype(np.float32)

